# revision 1
# baseline (speedup 1.0000x reference)
"""Birth-death interval loss on 8 trn2 NeuronCores.

Data-parallel over batch: core i handles batches [2i, 2i+2). Per (b,c) image:
the 512x512 f32 image is replicated into each 16-partition group's rows as 16
chunks of 16384 elements; one GPSIMD ap_gather fetches all 65536 endpoint
values (8192 indices per group, shared across the group's 16 channels); a
PE broadcast + fused DVE mask-multiply selects the correct chunk-channel per
point; a PE block-ones matmul collapses each group's 16 channels; DVE computes
(birth-death)^2 partial sums. Host sums the per-core partials.
"""
import os, sys, types

sys.path.insert(0, "/opt/trn_rl_repo")
sys.path.insert(0, "/root/.axon_site/trn_agent_boot")

import numpy as np


def _setup_env():
    import antenv  # noqa: F401

    if "antenv.axon_hooks" not in sys.modules:
        mod = types.ModuleType("antenv.axon_hooks")
        mod._hook = None
        mod.set_axon_ntff_profile_hook = lambda h: setattr(mod, "_hook", h)
        mod.get_axon_ntff_profile_hook = lambda: mod._hook
        sys.modules["antenv.axon_hooks"] = mod
        try:
            from trn_boot import _ntff_profile_via_ctypes

            mod._hook = _ntff_profile_via_ctypes("/opt/axon/libaxon_pjrt.so")
        except Exception:
            pass

    import concourse.tile as tile
    from concourse.vector_clock import ScopedClock
    from bass_rust import VectorClock

    def _split_drain_and_barrier(self, tick_clock, wait_clock):
        vals = list(tick_clock.global_clock)
        nz = [(i, v) for i, v in enumerate(vals) if v > 0]
        chunks = [nz[i : i + 1] for i in range(len(nz))] or [[]]
        for chunk in chunks:
            sub = [0] * len(vals)
            for i, v in chunk:
                sub[i] = v
            drain_inst = self.nc.sync.drain()
            wait_clock.add_sem_waits(
                drain_inst.ins, ScopedClock({None: VectorClock(sub)})
            )
        self.nc.all_engine_barrier()
        assert self.sems is not None
        popped = self.nc._tile_sem_poison_stack.pop()
        assert popped is self._sem_poison
        self.nc.clear_and_free_semaphores(list(self.sems.allocated().values()))
        self.nc.all_engine_barrier()

    tile.TileContext._drain_and_barrier = _split_drain_and_barrier


def _split_waits(nc):
    from concourse import mybir

    ctr = [0]
    for f in nc.m.functions:
        for bb in f.blocks:
            new = []
            changed = False
            for inst in bb.instructions:
                si = inst.sync_info
                if si is not None and len(si.on_wait) > 1:
                    waits = list(si.on_wait)
                    for w in waits[:-1]:
                        ctr[0] += 1
                        new.append(
                            mybir.InstEventSemaphore(
                                name=f"I-wsplit{ctr[0]}",
                                ins=[], outs=[], engine=inst.engine,
                                sync_info=mybir.SyncInfo(on_wait=[w], on_update=[]),
                            )
                        )
                    inst.sync_info = mybir.SyncInfo(
                        on_wait=waits[-1:], on_update=list(si.on_update)
                    )
                    changed = True
                new.append(inst)
            if changed:
                bb.instructions = new


_BUILT = None


def _build():
    global _BUILT
    if _BUILT is not None:
        return _BUILT
    _setup_env()
    import concourse.bass as bass
    import concourse.tile as tile
    from concourse import mybir, library_config
    from contextlib import ExitStack

    f32 = mybir.dt.float32
    i32 = mybir.dt.int32
    i16 = mybir.dt.int16
    Alu = mybir.AluOpType

    NIMG = 16  # 2 batches x 8 channels per core

    nc = bass.Bass("TRN2", target_bir_lowering=False, debug=False, num_devices=8)
    pred_d = nc.dram_tensor("pred", [2, 8, 512, 512], f32, kind="ExternalInput").ap()
    iv0_d = nc.dram_tensor("iv0", [2, 8, 16384, 8], i32, kind="ExternalInput").ap()
    iv1_d = nc.dram_tensor("iv1", [2, 8, 16384, 8], i32, kind="ExternalInput").ap()
    wbc_d = nc.dram_tensor("wbc", [128, 2048], f32, kind="ExternalInput").ap()
    wsel_d = nc.dram_tensor("wsel", [128, 8], f32, kind="ExternalInput").ap()
    iota_d = nc.dram_tensor("iota", [128, 1], f32, kind="ExternalInput").ap()
    out_d = nc.dram_tensor("out", [8, 512], f32, kind="ExternalOutput").ap()

    with tile.TileContext(nc) as tc, ExitStack() as ctx:
        cpool = ctx.enter_context(tc.tile_pool(name="c", bufs=1))
        imgp = ctx.enter_context(tc.tile_pool(name="img", bufs=2))
        recp = ctx.enter_context(tc.tile_pool(name="rec", bufs=1))
        idxp = ctx.enter_context(tc.tile_pool(name="idx", bufs=2))
        gop = ctx.enter_context(tc.tile_pool(name="go", bufs=1))
        mcp = ctx.enter_context(tc.tile_pool(name="mc", bufs=2))
        dp = ctx.enter_context(tc.tile_pool(name="d", bufs=2))
        psp = ctx.enter_context(tc.tile_pool(name="ps", bufs=4, space="PSUM"))
        psv = ctx.enter_context(tc.tile_pool(name="pv", bufs=4, space="PSUM"))

        wbc = cpool.tile([128, 2048], f32, tag="wbc")
        wsel = cpool.tile([128, 8], f32, tag="wsel")
        iota = cpool.tile([128, 1], f32, tag="iota")
        acc = cpool.tile([8, 512], f32, tag="acc")
        nc.sync.dma_start(wbc[:], wbc_d[:])
        nc.sync.dma_start(wsel[:], wsel_d[:])
        nc.sync.dma_start(iota[:], iota_d[:])
        nc.gpsimd.load_library(library_config.ap_gather)

        def emit_dma_prep(t):
            b, ch = t // 8, t % 8
            # replicated image: partitions 16g+c hold image chunk c
            img = imgp.tile([128, 16384], f32, tag="img")
            src_ = pred_d[b, ch].rearrange("(a r) w -> a (r w)", a=16)
            for g in range(8):
                nc.sync.dma_start(img[16 * g : 16 * g + 16, :], src_)

            # interval records: partition p holds recs [128p,128p+128)
            recs = recp.tile([128, 2048], i32, tag="recs")
            nc.sync.dma_start(
                recs[:, :1024],
                iv0_d[b, ch].rearrange("(p r) w -> p (r w)", p=128),
            )
            nc.sync.dma_start(
                recs[:, 1024:],
                iv1_d[b, ch].rearrange("(p r) w -> p (r w)", p=128),
            )

            rv = recs[:].rearrange("p (t r w) -> p t r w", t=2, w=8)
            flat = idxp.tile([128, 512], i32, tag="flat")
            fv = flat[:].rearrange("p (m two) -> p m two", two=2)
            for k in range(2):
                nc.vector.scalar_tensor_tensor(
                    fv[:, 128 * k : 128 * k + 128, 0],
                    rv[:, k, :, 0], 512, rv[:, k, :, 2],
                    op0=Alu.mult, op1=Alu.add)
                nc.vector.scalar_tensor_tensor(
                    fv[:, 128 * k : 128 * k + 128, 1],
                    rv[:, k, :, 4], 512, rv[:, k, :, 6],
                    op0=Alu.mult, op1=Alu.add)

            tmpa = idxp.tile([128, 512], i32, tag="tmpa")
            tmpb = idxp.tile([128, 512], i32, tag="tmpb")
            idx16 = idxp.tile([128, 512], i16, tag="idx16")
            chv = idxp.tile([128, 512], f32, tag="chv")
            nc.vector.tensor_scalar(tmpa[:], flat[:], 16383, None, op0=Alu.bitwise_and)
            nc.scalar.copy(idx16[:], tmpa[:])
            nc.vector.tensor_scalar(tmpb[:], flat[:], 14, None,
                                    op0=Alu.arith_shift_right)
            nc.scalar.copy(chv[:], tmpb[:])
            return img, idx16, chv

        state = emit_dma_prep(0)
        for t in range(NIMG):
            img, idx16, chv = state

            gouts = []
            for h in range(2):
                gout_h = gop.tile([128, 4096], f32, tag=f"gout{h}")
                nc.gpsimd.ap_gather(gout_h[:], img[:],
                                    idx16[:, 256 * h : 256 * h + 256],
                                    channels=128, num_elems=16384, d=1,
                                    num_idxs=4096)
                gouts.append(gout_h)

            # hoist next image's DMA + index prep ahead of the mask-select
            # burst so it runs at the DVE/ACT queue heads during the gathers
            if t + 1 < NIMG:
                state = emit_dma_prep(t + 1)

            for h in range(2):
                gv = gouts[h][:].rearrange("q (s p) -> q p s", p=16)
                co = 256 * h
                for pp in range(16):
                    chb = psp.tile([128, 256], f32, tag="chb")
                    nc.tensor.matmul(chb[:], wbc[:, 128 * pp : 128 * pp + 128],
                                     chv[:, co : co + 256], start=True, stop=True)
                    mc = mcp.tile([128, 256], f32, tag="mc")
                    nc.vector.scalar_tensor_tensor(
                        mc[:], chb[:], iota[:, 0:1], gv[:, pp, :],
                        op0=Alu.is_equal, op1=Alu.mult)
                    vals = psv.tile([8, 256], f32, tag="vals")
                    nc.tensor.matmul(vals[:], wsel[:], mc[:], start=True, stop=True)
                    sv = dp.tile([8, 256], f32, tag="sv")
                    nc.scalar.copy(sv[:], vals[:])
                    vv = sv[:].rearrange("y (m two) -> y m two", two=2)
                    dt_ = dp.tile([8, 128], f32, tag="dt")
                    nc.vector.tensor_sub(dt_[:], vv[:, :, 0], vv[:, :, 1])
                    dsq = dp.tile([8, 128], f32, tag="dsq")
                    nc.vector.scalar_tensor_tensor(
                        dsq[:], dt_[:], 1.0, dt_[:],
                        op0=Alu.mult, op1=Alu.mult,
                        accum_out=acc[:, 32 * t + 2 * pp + h : 32 * t + 2 * pp + h + 1])

        nc.sync.dma_start(out_d[:], acc[:])

    from concourse.library_overlay import lower_extended_insts

    lower_extended_insts(nc)
    _split_waits(nc)
    _BUILT = nc
    return nc


def kernel(prediction, intervals_comp_0, intervals_comp_1):
    nc = _build()
    from concourse.bass_utils import run_bass_kernel_spmd

    pred = np.ascontiguousarray(np.asarray(prediction, dtype=np.float32))
    iv0 = np.ascontiguousarray(np.asarray(intervals_comp_0)).astype(np.int64, copy=False)
    iv1 = np.ascontiguousarray(np.asarray(intervals_comp_1)).astype(np.int64, copy=False)
    iv0v = iv0.view(np.int32).reshape(16, 8, 16384, 8)
    iv1v = iv1.view(np.int32).reshape(16, 8, 16384, 8)

    qq = np.arange(128)
    pp_ = np.arange(2048)
    wbc = (qq[:, None] == (pp_[None, :] % 128 // 16) * 16 + pp_[None, :] // 128).astype(np.float32)
    wsel = (qq[:, None] // 16 == np.arange(8)[None, :]).astype(np.float32)
    iota = (qq % 16).astype(np.float32).reshape(128, 1)

    in_maps = []
    for i in range(8):
        in_maps.append({
            "pred": pred[2 * i : 2 * i + 2],
            "iv0": iv0v[2 * i : 2 * i + 2],
            "iv1": iv1v[2 * i : 2 * i + 2],
            "wbc": wbc, "wsel": wsel, "iota": iota,
        })

    trace = bool(int(os.environ.get("BDL_TRACE", "0")))
    res = run_bass_kernel_spmd(nc, in_maps, list(range(8)), trace=trace)
    if trace:
        print(f"HW exec time: {res.exec_time_ns} ns", flush=True)
        kernel.last_result = res

    total = np.float64(0.0)
    for i in range(8):
        total += np.asarray(res.results[i]["out"], dtype=np.float64).sum()
    return np.float32(total / 16.0)



# revision 6
# speedup vs baseline: 94.6784x; 94.6784x over previous
"""Birth-death interval loss on 8 trn2 NeuronCores.

Data-parallel over batch: core i handles batches [2i, 2i+2) = 16 images.
Each 16-partition group holds ONE full 512x512 f32 image as 16 chunks of
16384 (so 8 distinct images per SBUF generation, 2 generations) -- no
replicated-image DMA. Per call-pair, one GPSIMD ap_gather fetches 4096
birth endpoints per group and a second fetches the 4096 matching deaths.
A per-pp PE matmul broadcasts chunk codes (bf16, 1 cyc/row), one fused
DVE STT does the chunk-select mask for births+deaths together, and the
birth-death subtraction happens FREE in PSUM via +wsel / -wsel
accumulating matmuls. ACT squares + accumulates each PSUM bank into the
per-core partial sums. Host sums the per-core partials.
"""
import os, sys, types

sys.path.insert(0, "/opt/trn_rl_repo")
sys.path.insert(0, "/root/.axon_site/trn_agent_boot")

import numpy as np


def _setup_env():
    import antenv  # noqa: F401

    if "antenv.axon_hooks" not in sys.modules:
        mod = types.ModuleType("antenv.axon_hooks")
        mod._hook = None
        mod.set_axon_ntff_profile_hook = lambda h: setattr(mod, "_hook", h)
        mod.get_axon_ntff_profile_hook = lambda: mod._hook
        sys.modules["antenv.axon_hooks"] = mod
        try:
            from trn_boot import _ntff_profile_via_ctypes

            mod._hook = _ntff_profile_via_ctypes("/opt/axon/libaxon_pjrt.so")
        except Exception:
            pass

    import concourse.tile as tile
    from concourse.vector_clock import ScopedClock
    from bass_rust import VectorClock

    def _split_drain_and_barrier(self, tick_clock, wait_clock):
        vals = list(tick_clock.global_clock)
        nz = [(i, v) for i, v in enumerate(vals) if v > 0]
        chunks = [nz[i : i + 1] for i in range(len(nz))] or [[]]
        for chunk in chunks:
            sub = [0] * len(vals)
            for i, v in chunk:
                sub[i] = v
            drain_inst = self.nc.sync.drain()
            wait_clock.add_sem_waits(
                drain_inst.ins, ScopedClock({None: VectorClock(sub)})
            )
        self.nc.all_engine_barrier()
        assert self.sems is not None
        popped = self.nc._tile_sem_poison_stack.pop()
        assert popped is self._sem_poison
        self.nc.clear_and_free_semaphores(list(self.sems.allocated().values()))
        self.nc.all_engine_barrier()

    tile.TileContext._drain_and_barrier = _split_drain_and_barrier


def _split_waits(nc):
    from concourse import mybir

    ctr = [0]
    for f in nc.m.functions:
        for bb in f.blocks:
            new = []
            changed = False
            for inst in bb.instructions:
                si = inst.sync_info
                if si is not None and len(si.on_wait) > 1:
                    waits = list(si.on_wait)
                    for w in waits[:-1]:
                        ctr[0] += 1
                        new.append(
                            mybir.InstEventSemaphore(
                                name=f"I-wsplit{ctr[0]}",
                                ins=[], outs=[], engine=inst.engine,
                                sync_info=mybir.SyncInfo(on_wait=[w], on_update=[]),
                            )
                        )
                    inst.sync_info = mybir.SyncInfo(
                        on_wait=waits[-1:], on_update=list(si.on_update)
                    )
                    changed = True
                new.append(inst)
            if changed:
                bb.instructions = new


_BUILT = None

NPAIR = 8  # call-pairs per generation (256 records/partition each)


def _build():
    global _BUILT
    if _BUILT is not None:
        return _BUILT
    _setup_env()
    import concourse.bass as bass
    import concourse.tile as tile
    from concourse import mybir, library_config
    from contextlib import ExitStack

    f32 = mybir.dt.float32
    bf16 = mybir.dt.bfloat16
    i32 = mybir.dt.int32
    i16 = mybir.dt.int16
    Alu = mybir.AluOpType
    Act = mybir.ActivationFunctionType

    nc = bass.Bass("TRN2", target_bir_lowering=False, debug=False, num_devices=8)
    img_d = nc.dram_tensor("img", [2, 128, 16384], f32, kind="ExternalInput").ap()
    recs_d = nc.dram_tensor("recs", [2, 128, 8192], i16, kind="ExternalInput").ap()
    wbc_d = nc.dram_tensor("wbc", [128, 2048], bf16, kind="ExternalInput").ap()
    wsp_d = nc.dram_tensor("wsp", [128, 8], bf16, kind="ExternalInput").ap()
    wsn_d = nc.dram_tensor("wsn", [128, 8], bf16, kind="ExternalInput").ap()
    iota_d = nc.dram_tensor("iota", [128, 1], f32, kind="ExternalInput").ap()
    out_d = nc.dram_tensor("out", [8, 2 * NPAIR * 8], f32, kind="ExternalOutput").ap()

    with tile.TileContext(nc) as tc, ExitStack() as ctx:
        cpool = ctx.enter_context(tc.tile_pool(name="c", bufs=1))
        imgp = ctx.enter_context(tc.tile_pool(name="img", bufs=1))
        recp = ctx.enter_context(tc.tile_pool(name="rec", bufs=1))
        idxp = ctx.enter_context(tc.tile_pool(name="idx", bufs=2))
        gop = ctx.enter_context(tc.tile_pool(name="go", bufs=2))
        mcp = ctx.enter_context(tc.tile_pool(name="mc", bufs=2))
        sqp = ctx.enter_context(tc.tile_pool(name="sq", bufs=2))
        psb = ctx.enter_context(tc.tile_pool(name="pb", bufs=2, space="PSUM"))
        psv = ctx.enter_context(tc.tile_pool(name="pv", bufs=4, space="PSUM"))

        wbc = cpool.tile([128, 2048], bf16, tag="wbc")
        wsp = cpool.tile([128, 8], bf16, tag="wsp")
        wsn = cpool.tile([128, 8], bf16, tag="wsn")
        iota = cpool.tile([128, 1], f32, tag="iota")
        acc = cpool.tile([8, 2 * NPAIR * 8], f32, tag="acc")
        nc.sync.dma_start(wbc[:], wbc_d[:])
        nc.sync.dma_start(wsp[:], wsp_d[:])
        nc.sync.dma_start(wsn[:], wsn_d[:])
        nc.sync.dma_start(iota[:], iota_d[:])
        nc.gpsimd.load_library(library_config.ap_gather)

        img = imgp.tile([128, 16384], f32, tag="img")

        def emit_prep(recs, k):
            # records 256k..256k+256 per partition -> flat birth/death idx
            rsl = recs[:, 1024 * k : 1024 * k + 1024]
            rec32 = idxp.tile([128, 1024], i32, tag="rec32")
            nc.scalar.copy(rec32[:], rsl)
            rv = rec32[:].rearrange("p (r w) -> p r w", w=4)
            fv = idxp.tile([128, 512], i32, tag="fv")
            for e in range(2):  # birth, death
                nc.vector.scalar_tensor_tensor(
                    fv[:, 256 * e : 256 * e + 256],
                    rv[:, :, 2 * e], 512, rv[:, :, 2 * e + 1],
                    op0=Alu.mult, op1=Alu.add)
            tmpa = idxp.tile([128, 512], i32, tag="tmpa")
            idx16 = idxp.tile([128, 512], i16, tag="idx16")
            chv = idxp.tile([128, 512], bf16, tag="chv")
            nc.vector.tensor_scalar(tmpa[:], fv[:], 16383, None, op0=Alu.bitwise_and)
            nc.scalar.copy(idx16[:], tmpa[:])
            nc.vector.tensor_scalar(tmpa[:], fv[:], 14, None,
                                    op0=Alu.arith_shift_right)
            nc.scalar.copy(chv[:], tmpa[:])
            return idx16, chv

        for gen in range(2):
            nc.sync.dma_start(img[:], img_d[gen])
            recs = recp.tile([128, 8192], i16, tag="recs")
            nc.sync.dma_start(recs[:], recs_d[gen])

            state = emit_prep(recs, 0)
            for k in range(NPAIR):
                idx16, chv = state
                gout = gop.tile([128, 8192], f32, tag="gout")
                for e in range(2):
                    nc.gpsimd.ap_gather(gout[:, 4096 * e : 4096 * e + 4096],
                                        img[:], idx16[:, 256 * e : 256 * e + 256],
                                        channels=128, num_elems=16384, d=1,
                                        num_idxs=4096)
                if k + 1 < NPAIR:
                    state = emit_prep(recs, k + 1)

                # chunk-select mask for births+deaths in one STT per pp
                mc = mcp.tile([128, 8192], bf16, tag="mc")
                gv = gout[:].rearrange("q (e s p) -> q (e s) p", e=2, p=16)
                for pp in range(16):
                    chb = psb.tile([128, 512], f32, tag="chb")
                    nc.tensor.matmul(chb[:], wbc[:, 128 * pp : 128 * pp + 128],
                                     chv[:], start=True, stop=True)
                    nc.vector.scalar_tensor_tensor(
                        mc[:, 512 * pp : 512 * pp + 512],
                        chb[:], iota[:, 0:1], gv[:, :, pp],
                        op0=Alu.is_equal, op1=Alu.mult)

                # birth - death lands in PSUM via +wsel / -wsel accumulation
                mv = mc[:].rearrange("q (t e s) -> q t e s", t=16, e=2)
                for half in range(2):
                    vbanks = []
                    for b in range(4):
                        pp0 = 8 * half + 2 * b
                        dt_ = psv.tile([8, 512], f32, tag="dt")
                        # full-bank write: one start=True per PSUM zero region
                        nc.tensor.matmul(dt_[:], wsp[:],
                                         mv[:, pp0 : pp0 + 2, 0, :],
                                         start=True, stop=False)
                        vbanks.append((dt_, pp0))
                    for dt_, pp0 in vbanks:
                        nc.tensor.matmul(dt_[:], wsn[:],
                                         mv[:, pp0 : pp0 + 2, 1, :],
                                         start=False, stop=True)
                    for b, (dt_, pp0) in enumerate(vbanks):
                        sq = sqp.tile([8, 512], f32, tag="sq")
                        col = (gen * NPAIR + k) * 8 + half * 4 + b
                        nc.scalar.activation(sq[:], dt_[:], Act.Square,
                                             accum_out=acc[:, col : col + 1])

        nc.sync.dma_start(out_d[:], acc[:])

    from concourse.library_overlay import lower_extended_insts

    lower_extended_insts(nc)
    _split_waits(nc)
    _BUILT = nc
    return nc


def _host_arrange(prediction, intervals_comp_0, intervals_comp_1):
    pred = np.ascontiguousarray(np.asarray(prediction, dtype=np.float32))
    # img[core, gen, 16g+c, :] = image (2*core+gen, ch=g) chunk c
    img = pred.reshape(8, 2, 8, 16, 16384).reshape(8, 2, 128, 16384)

    iv0 = np.asarray(intervals_comp_0).astype(np.int16)
    iv1 = np.asarray(intervals_comp_1).astype(np.int16)
    # per image: concat(iv0, iv1) -> 32768 records of (y0,x0,y1,x1)
    ivcat = np.concatenate(
        [iv0.reshape(16, 8, 16384, 4), iv1.reshape(16, 8, 16384, 4)], axis=2
    )  # [b, ch, 32768, 4]
    # partition 16g+p <- records [2048p, 2048(p+1)) of image (b=2*core+gen, ch=g)
    recs = ivcat.reshape(8, 2, 8, 16, 2048 * 4).reshape(8, 2, 128, 8192)
    return img, np.ascontiguousarray(recs)


def kernel(prediction, intervals_comp_0, intervals_comp_1):
    nc = _build()
    from concourse.bass_utils import run_bass_kernel_spmd

    img, recs = _host_arrange(prediction, intervals_comp_0, intervals_comp_1)

    qq = np.arange(128)
    pp_ = np.arange(2048)
    import ml_dtypes

    wbc = (qq[:, None] == (pp_[None, :] % 128 // 16) * 16 + pp_[None, :] // 128)
    wbc = wbc.astype(ml_dtypes.bfloat16)
    wsp = (qq[:, None] // 16 == np.arange(8)[None, :]).astype(ml_dtypes.bfloat16)
    wsn = (-wsp.astype(np.float32)).astype(ml_dtypes.bfloat16)
    iota = (qq % 16).astype(np.float32).reshape(128, 1)

    in_maps = []
    for i in range(8):
        in_maps.append({
            "img": img[i], "recs": recs[i],
            "wbc": wbc, "wsp": wsp, "wsn": wsn, "iota": iota,
        })

    trace = bool(int(os.environ.get("BDL_TRACE", "0")))
    res = run_bass_kernel_spmd(nc, in_maps, list(range(8)), trace=trace)
    if trace:
        print(f"HW exec time: {res.exec_time_ns} ns", flush=True)
        kernel.last_result = res

    total = np.float64(0.0)
    for i in range(8):
        total += np.asarray(res.results[i]["out"], dtype=np.float64).sum()
    return np.float32(total / 16.0)
